# revision 1
# baseline (speedup 1.0000x reference)
"""Data-parallel GAT-module kernel for 8 Trainium2 NeuronCores.

Shards batch N=64 across the 8 cores (8 samples per core); A and all
1x1-conv weights are replicated (tiny). No cross-device communication in
the forward pass. Accepts FULL unsharded inputs, returns the FULL output.

Hardcoded problem shapes: x (64, 64, 256, 25), A (25, 25),
Wq/Wk (8, 64), Wv (64, 64), Wr (64, 8).
"""
import numpy as np
import jax
import jax.numpy as jnp
from functools import partial

N, C, T, V = 64, 64, 256, 25
H, O = 8, 64
N_CORES = 8

_fwd_cache = {}


def _forward(x, A, alpha, Wq, bq, Wk, bk, Wv, bv, Wr, br):
    # x: (N/8, C, T, V) local shard
    x_mean = x.mean(axis=2)                                           # (n, C, V)
    q = jnp.einsum('ncv,hc->nhv', x_mean, Wq) + bq[None, :, None]     # (n, H, V)
    k = jnp.einsum('ncv,hc->nhv', x_mean, Wk) + bk[None, :, None]     # (n, H, V)
    v = jnp.einsum('nctv,oc->notv', x, Wv) + bv[None, :, None, None]  # (n, O, T, V)
    attn = jnp.tanh(q[:, :, :, None] - k[:, :, None, :])              # (n, H, V, V)
    rep = jnp.einsum('nhuv,oh->nouv', attn, Wr) + br[None, :, None, None]
    masked = alpha * rep + A[None, None, :, :]                        # (n, O, V, V)
    out = jnp.einsum('ncuv,nctv->nctu', masked, v)                    # (n, O, T, V)
    return out


def _get_pmapped(n_dev):
    if n_dev not in _fwd_cache:
        _fwd_cache[n_dev] = jax.pmap(
            _forward,
            in_axes=(0, None, None, None, None, None, None, None, None, None, None),
            devices=jax.devices()[:n_dev],
        )
    return _fwd_cache[n_dev]


def kernel(x, A, alpha, Wq, bq, Wk, bk, Wv, bv, Wr, br):
    x = np.asarray(x, dtype=np.float32)
    n_dev = min(N_CORES, jax.device_count())
    per = N // n_dev
    xs = x.reshape(n_dev, per, C, T, V)
    f = _get_pmapped(n_dev)
    out = f(
        xs,
        jnp.asarray(A, jnp.float32),
        jnp.asarray(alpha, jnp.float32),
        jnp.asarray(Wq, jnp.float32),
        jnp.asarray(bq, jnp.float32),
        jnp.asarray(Wk, jnp.float32),
        jnp.asarray(bk, jnp.float32),
        jnp.asarray(Wv, jnp.float32),
        jnp.asarray(bv, jnp.float32),
        jnp.asarray(Wr, jnp.float32),
        jnp.asarray(br, jnp.float32),
    )
    out = np.asarray(out).reshape(N, O, T, V).astype(np.float32)
    return out

